# revision 33
# baseline (speedup 1.0000x reference)
"""ChildSumTreeLSTM on 8 trn2 NeuronCores — v2.

Tree = reversed complete 4-ary heap (id = N-1-heap; heap j's children are
4j+1..4j+4).  Shard the 16 depth-4 subtrees rooted at heap 5..20 as pairs
across 8 cores (core c owns heap {5+2c, 6+2c}).  Per core the subtree pair is
a uniform padded forest with levels of 512 (leaf) / 128 (L5) / 32 (L4) /
8 (L3) / 2 (T2) slots; an 8KB AllGather moves the 16 T2 roots everywhere and
every core redundantly computes T1 (heap 1..4) and T0 (heap 0).

Layouts:
 - "T layout" (mem on partitions, nodes on free axis) for the wide levels:
   phase-A X GEMM, leaf elementwise, and the L5 forget-gate path.
 - "N layout" (nodes on partitions, mem on free axis) for everything narrow:
   per-level GEMMs become a handful of 512-wide matmuls (stationary = chsT /
   transposed h), X+bias contributions are folded into the PSUM accumulation
   with selector/replicator matmuls, and gate activations read PSUM directly.
 - child k of parent j always sits at slot k*n_par + j, so child-group sums
   are plain partition-range (N) or free-range (T) adds.
"""

import os
import sys

sys.path.insert(0, "/opt/trn_rl_repo")

import numpy as np

import concourse.bass as bass
import concourse.bacc as bacc
import concourse.mybir as mybir
import concourse.tile as tile
from concourse.bass_utils import run_bass_kernel_spmd

F32 = mybir.dt.float32
F16 = mybir.dt.float16  # GEMM operand dtype (fp16: single-pass PE, 10-bit mantissa)
AF = mybir.ActivationFunctionType
ALU = mybir.AluOpType
AX = mybir.AxisListType

N = 4096
MEM = 512
IN_DIM = 512
NCORES = 8
P = 128
KT = 4  # contraction tiles (512 / 128)

NL6, NL5, NL4, NL3, NT2 = 512, 128, 32, 8, 2
XCOLS = NL6 + NL5 + 47  # 687: leaf | L5 | N-rows (L4 32, L3 8, T2 2, T1 4, T0 1)
NROWS = 47
R4_OFF, R3_OFF, R2_OFF, R1_OFF, R0_OFF = 0, 32, 40, 42, 46

LAST_RESULT = None  # BassKernelResults of the most recent run (for test.py)

N_WARM = int(os.environ.get("KERNEL_WARM", "28"))  # keep-PE-warm matmuls during CC
DEBUG = bool(int(os.environ.get("KERNEL_DEBUG", "0")))


def _core_heaps(c):
    # child k of parent j sits at slot 4j+k, at every level
    t2h = np.array([5 + 2 * c, 6 + 2 * c])
    l3h = np.array([4 * t2h[j // 4] + 1 + j % 4 for j in range(NL3)])
    l4h = np.array([4 * l3h[s // 4] + 1 + s % 4 for s in range(NL4)])
    l5h = np.array([4 * l4h[v // 4] + 1 + v % 4 for v in range(NL5)])
    l6h = np.array([4 * l5h[u // 4] + 1 + u % 4 for u in range(NL6)])
    nrows = np.concatenate([l4h, l3h, t2h, [1, 2, 3, 4], [0]])
    return l6h, np.concatenate([l6h.clip(max=N), l5h, nrows])  # col heaps (l6 clipped -> pad)


def _build_program():
    nc = bacc.Bacc("TRN2", target_bir_lowering=False, debug=False)

    xin_d = nc.dram_tensor("xin", [IN_DIM, XCOLS], F16, kind="ExternalInput")
    wx_d = nc.dram_tensor("wx", [IN_DIM, 4 * MEM], F16, kind="ExternalInput")
    ws_d = nc.dram_tensor("ws", [MEM, 3 * MEM], F16, kind="ExternalInput")
    wf_d = nc.dram_tensor("wf", [MEM, MEM], F16, kind="ExternalInput")
    lbi_d = nc.dram_tensor("lbi", [P, 12], F32, kind="ExternalInput")
    lbf_d = nc.dram_tensor("lbf", [P, 4], F32, kind="ExternalInput")
    nb_d = nc.dram_tensor("nbias", [1, 4 * MEM], F16, kind="ExternalInput")
    r32_d = nc.dram_tensor("r32", [32, 128], F16, kind="ExternalInput")
    r8_d = nc.dram_tensor("r8", [8, 32], F16, kind="ExternalInput")
    r2_d = nc.dram_tensor("r2", [2, 8], F16, kind="ExternalInput")
    r4_d = nc.dram_tensor("r4", [4, 16], F16, kind="ExternalInput")
    r1_d = nc.dram_tensor("r1", [1, 4], F16, kind="ExternalInput")
    s32_d = nc.dram_tensor("s32", [128, 32], F32, kind="ExternalInput")
    s8_d = nc.dram_tensor("s8", [32, 8], F32, kind="ExternalInput")
    s2_d = nc.dram_tensor("s2", [8, 2], F32, kind="ExternalInput")
    s4_d = nc.dram_tensor("s4", [16, 4], F32, kind="ExternalInput")
    s1_d = nc.dram_tensor("s1", [4, 1], F32, kind="ExternalInput")
    id16_d = nc.dram_tensor("id16", [P, P], F16, kind="ExternalInput")
    id32_d = nc.dram_tensor("id32", [P, P], F32, kind="ExternalInput")
    ones_d = nc.dram_tensor("ones", [1, P], F16, kind="ExternalInput")
    cm_d = nc.dram_tensor("cmask", [P, NL6], F32, kind="ExternalInput")
    out_d = nc.dram_tensor("out", [1, MEM], F32, kind="ExternalOutput")
    dbg = {}
    if DEBUG:
        for nm, shape, dt in [("xt0", [P, NL6], F16), ("h3", [P, NL6], F16),
                              ("c3", [P, NL6], F32), ("xn", [NROWS, 4 * MEM], F16),
                              ("hn5", [P, MEM], F16), ("cn5", [P, MEM], F32),
                              ("hn4", [NL4, MEM], F16), ("cn4", [NL4, MEM], F32),
                              ("hn2", [NT2, MEM], F32), ("cn2", [NT2, MEM], F32),
                              ("hg", [16, MEM], F32), ("cg", [16, MEM], F32),
                              ("hn1", [4, MEM], F16), ("xf5", [P, MEM], F16)]:
            dbg[nm] = nc.dram_tensor(f"dbg_{nm}", shape, dt, kind="ExternalOutput")
    contrib_d = nc.dram_tensor("contrib", [NT2, 2 * MEM], F32)
    gath_d = nc.dram_tensor("gath", [NCORES * NT2, 2 * MEM], F32, addr_space="Shared")

    with tile.TileContext(nc) as tc:
        with (
            tc.tile_pool(name="wpool", bufs=1) as wpool,
            tc.tile_pool(name="xpool", bufs=1) as xpool,
            tc.tile_pool(name="state", bufs=1) as state,
            tc.tile_pool(name="tmp", bufs=1) as tmp,
            tc.tile_pool(name="psA", bufs=2, space="PSUM") as psA,
            tc.tile_pool(name="psF", bufs=2, space="PSUM") as psF,
            tc.tile_pool(name="psI", bufs=3, space="PSUM") as psI,
            tc.tile_pool(name="psT", bufs=1, space="PSUM") as psT,
        ):
            # ---- loads (phase-A inputs first so X GEMM starts early) ----
            wx_s = [wpool.tile([P, 4 * MEM], F16, name="t", tag=f"wx{k}") for k in range(KT)]
            in_s = [wpool.tile([P, XCOLS], F16, name="t", tag=f"in{k}") for k in range(KT)]
            ws_s = [wpool.tile([P, 3 * MEM], F16, name="t", tag=f"ws{k}") for k in range(KT)]
            wf_s = [wpool.tile([P, MEM], F16, name="t", tag=f"wf{k}") for k in range(KT)]
            for k in range(KT):
                r = slice(k * P, (k + 1) * P)
                nc.sync.dma_start(wx_s[k][:], wx_d[r, :])
                nc.sync.dma_start(in_s[k][:], xin_d[r, :])
            for k in range(KT):
                r = slice(k * P, (k + 1) * P)
                nc.sync.dma_start(ws_s[k][:], ws_d[r, :])
                nc.sync.dma_start(wf_s[k][:], wf_d[r, :])
            lbi_s = wpool.tile([P, 12], F32, name="t", tag="lbi")
            lbf_s = wpool.tile([P, 4], F32, name="t", tag="lbf")
            nb_s = wpool.tile([1, 4 * MEM], F16, name="t", tag="nb")
            r32_s = wpool.tile([32, 128], F16, name="t", tag="r32")
            r8_s = wpool.tile([8, 32], F16, name="t", tag="r8")
            r2_s = wpool.tile([2, 8], F16, name="t", tag="r2")
            r4_s = wpool.tile([4, 16], F16, name="t", tag="r4")
            r1_s = wpool.tile([1, 4], F16, name="t", tag="r1")
            s32_s = wpool.tile([128, 32], F32, name="t", tag="s32")
            s8_s = wpool.tile([32, 8], F32, name="t", tag="s8")
            s2_s = wpool.tile([8, 2], F32, name="t", tag="s2")
            s4_s = wpool.tile([16, 4], F32, name="t", tag="s4")
            s1_s = wpool.tile([4, 1], F32, name="t", tag="s1")
            id16_s = wpool.tile([P, P], F16, name="t", tag="id16")
            id32_s = wpool.tile([P, P], F32, name="t", tag="id32")
            ones_s = wpool.tile([1, P], F16, name="t", tag="ones")
            cm_s = wpool.tile([P, NL6], F32, name="t", tag="cm")
            for t_, d_ in [(lbi_s, lbi_d), (lbf_s, lbf_d), (nb_s, nb_d),
                           (r32_s, r32_d), (r8_s, r8_d), (r2_s, r2_d),
                           (r4_s, r4_d), (r1_s, r1_d), (id16_s, id16_d),
                           (id32_s, id32_d), (ones_s, ones_d), (cm_s, cm_d),
                           (s32_s, s32_d), (s8_s, s8_d), (s2_s, s2_d),
                           (s4_s, s4_d), (s1_s, s1_d)]:
                nc.sync.dma_start(t_[:], d_[:])

            # ---- phase A-T: X for the 512 leaf cols, [16 gate-tiles, 512] fp16 ----
            Xt = [xpool.tile([P, NL6], F16, name="t", tag=f"X{mc}") for mc in range(16)]
            for mc in range(16):
                ps = psA.tile([P, MEM], F32, name="t", tag="a")
                for k in range(KT):
                    nc.tensor.matmul(ps[:], wx_s[k][:, mc * P:(mc + 1) * P],
                                     in_s[k][:, 0:NL6],
                                     start=(k == 0), stop=(k == KT - 1))
                nc.vector.tensor_copy(Xt[mc][:], ps[:])

            # ---- phase A-N: X_N for the 47 narrow rows (bias folded in) ----
            xn = xpool.tile([NROWS, 4 * MEM], F16, name="t", tag="xn")
            for ch in range(4):
                cs = slice(ch * MEM, (ch + 1) * MEM)
                ps = psI.tile([P, MEM], F32, name="t", tag="i")
                for k in range(KT):
                    nc.tensor.matmul(ps[0:NROWS, :], in_s[k][:, NL6 + NL5:XCOLS],
                                     wx_s[k][:, cs], start=(k == 0), stop=False)
                nc.tensor.matmul(ps[0:NROWS, :], ones_s[:, 0:NROWS], nb_s[:, cs],
                                 start=False, stop=True)
                nc.vector.tensor_copy(xn[:, cs], ps[0:NROWS, :])
            # matmul operands must share a base partition: re-base each level's
            # X rows to partition 0 with sbuf->sbuf DMAs
            xn3 = xpool.tile([NL3, 4 * MEM], F16, name="t", tag="xn3")
            xn2 = xpool.tile([NT2, 4 * MEM], F16, name="t", tag="xn2")
            xn1 = xpool.tile([4, 4 * MEM], F16, name="t", tag="xn1")
            xn0 = xpool.tile([1, 4 * MEM], F16, name="t", tag="xn0")
            nc.sync.dma_start(xn3[:], xn[R3_OFF:R3_OFF + NL3, :])
            nc.sync.dma_start(xn2[:], xn[R2_OFF:R2_OFF + NT2, :])
            nc.sync.dma_start(xn1[:], xn[R1_OFF:R1_OFF + 4, :])
            nc.sync.dma_start(xn0[:], xn[R0_OFF:R0_OFF + 1, :])

            # ---- leaf step (T layout) ----
            H3 = [state.tile([P, NL6], F16, name="t", tag=f"H3{m}") for m in range(KT)]
            C3 = [state.tile([P, NL6], F32, name="t", tag=f"C3{m}") for m in range(KT)]
            for m in range(KT):
                ig = tmp.tile([P, NL6], F32, name="t", tag="lf_i", bufs=2)
                og = tmp.tile([P, NL6], F32, name="t", tag="lf_o", bufs=2)
                ug = tmp.tile([P, NL6], F32, name="t", tag="lf_u", bufs=2)
                nc.scalar.activation(ig[:], Xt[m][:], AF.Sigmoid, bias=lbi_s[:, m:m + 1])
                nc.scalar.activation(og[:], Xt[8 + m][:], AF.Sigmoid, bias=lbi_s[:, 4 + m:5 + m])
                nc.scalar.activation(ug[:], Xt[12 + m][:], AF.Tanh, bias=lbi_s[:, 8 + m:9 + m])
                cr = tmp.tile([P, NL6], F32, name="t", tag="lf_c", bufs=2)
                nc.vector.tensor_mul(cr[:], ig[:], ug[:])
                nc.vector.tensor_mul(C3[m][:], cr[:], cm_s[:])  # zero pad slots
                th = tmp.tile([P, NL6], F32, name="t", tag="lf_t", bufs=2)
                nc.scalar.activation(th[:], C3[m][:], AF.Tanh)
                nc.vector.tensor_mul(H3[m][:], og[:], th[:])

            # ---- L5 step (hybrid): f-path in T, iou + state in N ----
            # f-path: psF[m] = Wf.T@H3 + (Wxf.T@x_L5 broadcast x4); f=sig(+bias)
            xf5ps = psI.tile([P, MEM], F32, name="t", tag="i")
            for m in range(KT):
                for k in range(KT):
                    nc.tensor.matmul(xf5ps[:, m * P:(m + 1) * P],
                                     wx_s[k][:, (4 + m) * P:(5 + m) * P],
                                     in_s[k][:, NL6:NL6 + NL5],
                                     start=(k == 0), stop=(k == KT - 1))
            xf5 = tmp.tile([P, MEM], F16, name="t", tag="xf5")
            nc.vector.tensor_copy(xf5[:], xf5ps[:])

            fsN = psA.tile([P, MEM], F32, name="t", tag="a")
            for m in range(KT):
                ps = psF.tile([P, NL6], F32, name="t", tag="f")
                for k in range(KT):
                    nc.tensor.matmul(ps[:], wf_s[k][:, m * P:(m + 1) * P], H3[k][:],
                                     start=(k == 0), stop=(k == KT - 1))
                fx = xf5[:, m * P:(m + 1) * P]
                fxb = bass.AP(tensor=fx.tensor, offset=fx.offset, ap=list(fx.ap) + [[0, 4]])
                pv = ps[:].rearrange("p (n g) -> p n g", g=4)
                nc.vector.tensor_add(pv, pv, fxb)
                fg = tmp.tile([P, NL6], F32, name="t", tag="l5_f", bufs=2)
                nc.scalar.activation(fg[:], ps[:], AF.Sigmoid, bias=lbf_s[:, m:m + 1])
                fcc = tmp.tile([P, NL6], F32, name="t", tag="l5_fcc", bufs=2)
                nc.vector.tensor_mul(fcc[:], fg[:], C3[m][:])
                fsm = tmp.tile([P, NL5], F32, name="t", tag=f"l5_fs{m}")
                nc.vector.tensor_reduce(fsm[:], fcc[:].rearrange("p (n g) -> p n g", g=4),
                                        axis=AX.X, op=ALU.add)
                # transpose fs [128m, 128 nodes] -> fsN psum [128 nodes, 512]
                nc.tensor.transpose(fsN[:, m * P:(m + 1) * P], fsm[:], id32_s[:])

            # chsT (leaf child-h sums, contiguous groups of 4), fp16
            chsT5 = [tmp.tile([P, NL5], F16, name="t", tag=f"chs5{k}") for k in range(KT)]
            with nc.allow_low_precision(reason="sum of 4 fp16 h values"):
                for k in range(KT):
                    nc.vector.tensor_reduce(chsT5[k][:],
                                            H3[k][:].rearrange("p (n g) -> p n g", g=4),
                                            axis=AX.X, op=ALU.add)
            # iou-N: psum[128 L5 rows, 512] = Ws.T@chs + Wx.T@x + bias
            gi5 = tmp.tile([P, MEM], F32, name="t", tag="gi5")
            go5 = tmp.tile([P, MEM], F32, name="t", tag="go5")
            gu5 = tmp.tile([P, MEM], F32, name="t", tag="gu5")
            for gt, wso, xo in [(gi5, 0, 0), (go5, MEM, 2 * MEM), (gu5, 2 * MEM, 3 * MEM)]:
                ps = psI.tile([P, MEM], F32, name="t", tag="i")
                for k in range(KT):
                    nc.tensor.matmul(ps[:], chsT5[k][:], ws_s[k][:, wso:wso + MEM],
                                     start=(k == 0), stop=False)
                for k in range(KT):
                    nc.tensor.matmul(ps[:], in_s[k][:, NL6:NL6 + NL5],
                                     wx_s[k][:, xo:xo + MEM], start=False, stop=False)
                nc.tensor.matmul(ps[:], ones_s[:, 0:NL5], nb_s[:, xo:xo + MEM],
                                 start=False, stop=True)
                nc.scalar.activation(gt[:], ps[:], AF.Tanh if gt is gu5 else AF.Sigmoid)
            iu5 = tmp.tile([P, MEM], F32, name="t", tag="iu5")
            nc.vector.tensor_mul(iu5[:], gi5[:], gu5[:])
            cn5 = state.tile([P, MEM], F32, name="t", tag="cn5")
            nc.vector.tensor_add(cn5[:], iu5[:], fsN[:])
            th5 = tmp.tile([P, MEM], F32, name="t", tag="th5")
            nc.scalar.activation(th5[:], cn5[:], AF.Tanh)
            hn5 = tmp.tile([P, MEM], F16, name="t", tag="hn5")
            nc.vector.tensor_mul(hn5[:], go5[:], th5[:])
            hT5 = [state.tile([P, NL5], F16, name="t", tag=f"hT5{m}") for m in range(KT)]
            for m in range(KT):
                pt = psT.tile([P, P], F16, name="t", tag="t")
                nc.tensor.transpose(pt[:], hn5[:, m * P:(m + 1) * P], id16_s[:])
                nc.vector.tensor_copy(hT5[m][:], pt[:])

            # ---- uniform N-layout level ----
            def n_level(np_, hT_c, cN_c, xnl, R, S, hname, h_dtype=F16,
                        transpose_h=True):
                nch = 4 * np_
                # forget path: psum[nch, 512] = Wf.T@h_c + R.T@Xf (Xf carries bias)
                psf = psF.tile([P, MEM], F32, name="t", tag="f")
                for k in range(KT):
                    nc.tensor.matmul(psf[0:nch, :], hT_c[k][:, 0:nch], wf_s[k][:],
                                     start=(k == 0), stop=False)
                nc.tensor.matmul(psf[0:nch, :], R[0:np_, 0:nch],
                                 xnl[:, MEM:2 * MEM], start=False, stop=True)
                fg = tmp.tile([P, MEM], F32, name="t", tag="nl_f")
                nc.scalar.activation(fg[0:nch, :], psf[0:nch, :], AF.Sigmoid)
                fcc = tmp.tile([P, MEM], F32, name="t", tag="nl_fcc")
                nc.vector.tensor_mul(fcc[0:nch, :], fg[0:nch, :], cN_c[0:nch, :])
                # child-group sum over partitions via 0/1 matrix (fp32 matmul)
                psfs = psF.tile([P, MEM], F32, name="t", tag="f")
                nc.tensor.matmul(psfs[0:np_, :], S[0:nch, 0:np_], fcc[0:nch, :],
                                 start=True, stop=True)
                # chsT: grouped child-h sums along the free axis of hT_c
                chsT = [tmp.tile([P, np_], F16, name="t", tag=f"nl_ch{k}") for k in range(KT)]
                with nc.allow_low_precision(reason="sum of 4 fp16 h values"):
                    for k in range(KT):
                        nc.vector.tensor_reduce(
                            chsT[k][:],
                            hT_c[k][:, 0:nch].rearrange("p (j g) -> p j g", g=4),
                            axis=AX.X, op=ALU.add)
                # iou chunks (X_N fold carries Wx part + bias)
                gates = []
                for ci, (wso, xo) in enumerate([(0, 0), (MEM, 2 * MEM), (2 * MEM, 3 * MEM)]):
                    ps = psI.tile([P, MEM], F32, name="t", tag="i")
                    for k in range(KT):
                        nc.tensor.matmul(ps[0:np_, :], chsT[k][:], ws_s[k][:, wso:wso + MEM],
                                         start=(k == 0), stop=False)
                    nc.tensor.matmul(ps[0:np_, :], id16_s[0:np_, 0:np_],
                                     xnl[:, xo:xo + MEM], start=False, stop=True)
                    gt = tmp.tile([P, MEM], F32, name="t", tag=f"nl_g{ci}")
                    nc.scalar.activation(gt[0:np_, :], ps[0:np_, :],
                                         AF.Tanh if ci == 2 else AF.Sigmoid)
                    gates.append(gt)
                iu = tmp.tile([P, MEM], F32, name="t", tag="nl_iu")
                nc.vector.tensor_mul(iu[0:np_, :], gates[0][0:np_, :], gates[2][0:np_, :])
                cN = state.tile([np_, MEM], F32, name="t", tag=f"cN_{hname}")
                nc.vector.tensor_add(cN[:], iu[0:np_, :], psfs[0:np_, :])
                th = tmp.tile([P, MEM], F32, name="t", tag="nl_th")
                nc.scalar.activation(th[0:np_, :], cN[:], AF.Tanh)
                hN = state.tile([np_, MEM], h_dtype, name="t", tag=f"hN_{hname}")
                nc.vector.tensor_mul(hN[:], gates[1][0:np_, :], th[0:np_, :])
                hT = None
                if transpose_h:
                    hT = [state.tile([P, np_], F16, name="t", tag=f"hT_{hname}{m}")
                          for m in range(KT)]
                    for m in range(KT):
                        pt = psT.tile([P, P], F16, name="t", tag="t")
                        nc.tensor.transpose(pt[:, 0:np_], hN[:, m * P:(m + 1) * P],
                                            id16_s[0:np_, 0:np_])
                        nc.vector.tensor_copy(hT[m][:], pt[:, 0:np_])
                return hT, cN, hN

            hT4, cN4, hN4 = n_level(NL4, hT5, cn5, xn[0:NL4, :], r32_s[:], s32_s[:], "L4")
            hT3, cN3, _ = n_level(NL3, hT4, cN4, xn3[:], r8_s[:], s8_s[:], "L3")
            _, cN2, hN2 = n_level(NT2, hT3, cN3, xn2[:], r2_s[:], s2_s[:], "T2",
                                  h_dtype=F32, transpose_h=False)
            if DEBUG:
                nc.sync.dma_start(dbg["xt0"][:], Xt[0][:])
                nc.sync.dma_start(dbg["h3"][:], H3[0][:])
                nc.sync.dma_start(dbg["c3"][:], C3[0][:])
                nc.sync.dma_start(dbg["xn"][:], xn[:])
                nc.sync.dma_start(dbg["xf5"][:], xf5[:])
                nc.sync.dma_start(dbg["hn5"][:], hn5[:])
                nc.sync.dma_start(dbg["cn5"][:], cn5[:])
                nc.sync.dma_start(dbg["hn4"][:], hN4[:])
                nc.sync.dma_start(dbg["cn4"][:], cN4[:])
                nc.sync.dma_start(dbg["hn2"][:], hN2[:])
                nc.sync.dma_start(dbg["cn2"][:], cN2[:])

            # ---- 8KB AllGather of the 16 T2 roots ----
            nc.sync.dma_start(contrib_d[:, 0:MEM], hN2[:])
            nc.sync.dma_start(contrib_d[:, MEM:2 * MEM], cN2[:])
            nc.gpsimd.collective_compute(
                "AllGather", ALU.bypass,
                replica_groups=[list(range(NCORES))],
                ins=[contrib_d[:]],
                outs=[gath_d[:]],
            )
            # keep the PE HAM un-throttled across the collective wait
            if N_WARM:
                pw = psT.tile([P, P], F16, name="t", tag="t")
                for _ in range(N_WARM):
                    nc.tensor.transpose(pw[:], id16_s[:], id16_s[:])

            # gathered row r = heap-5 = 4j+k: already contiguous child groups
            hg = state.tile([16, MEM], F32, name="t", tag="hg")
            cg = state.tile([16, MEM], F32, name="t", tag="cg")
            nc.sync.dma_start(hg[:], gath_d[:, 0:MEM])
            nc.sync.dma_start(cg[:], gath_d[:, MEM:2 * MEM])
            hg16 = tmp.tile([16, MEM], F16, name="t", tag="hg16")
            nc.vector.tensor_copy(hg16[:], hg[:])
            hT2g = [state.tile([P, 16], F16, name="t", tag=f"hT2g{m}") for m in range(KT)]
            for m in range(KT):
                pt = psT.tile([P, P], F16, name="t", tag="t")
                nc.tensor.transpose(pt[:, 0:16], hg16[:, m * P:(m + 1) * P],
                                    id16_s[0:16, 0:16])
                nc.vector.tensor_copy(hT2g[m][:], pt[:, 0:16])

            hT1, cN1, hN1 = n_level(4, hT2g, cg, xn1[:], r4_s[:], s4_s[:], "T1")
            _, _, hN0 = n_level(1, hT1, cN1, xn0[:], r1_s[:], s1_s[:], "T0",
                                h_dtype=F32, transpose_h=False)
            if DEBUG:
                nc.sync.dma_start(dbg["hg"][:], hg[:])
                nc.sync.dma_start(dbg["cg"][:], cg[:])
                nc.sync.dma_start(dbg["hn1"][:], hN1[:])
            nc.sync.dma_start(out_d[:], hN0[:])

    nc.compile()
    return nc


_NC_CACHE = None


def kernel(inputs, Wx, bx, Ws, bs, Wf, bf, children):
    global LAST_RESULT, _NC_CACHE
    inputs = np.asarray(inputs, np.float32)
    Wx = np.asarray(Wx, np.float32)
    bx = np.asarray(bx, np.float32)
    Ws = np.asarray(Ws, np.float32)
    bs = np.asarray(bs, np.float32)
    Wf = np.asarray(Wf, np.float32)
    bf = np.asarray(bf, np.float32)

    Wx_h = Wx.astype(np.float16)
    Ws_h = Ws.astype(np.float16)
    Wf_h = Wf.astype(np.float16)
    # columns: i m0..3, o m0..3, u m0..3
    lbi = np.ascontiguousarray(np.concatenate(
        [(bx[g * 512 + m * P:(g * 512) + (m + 1) * P] +
          b2[m * P:(m + 1) * P])[:, None]
         for g, b2 in [(0, bs[0:512]), (2, bs[512:1024]), (3, bs[1024:1536])]
         for m in range(4)], axis=1), np.float32)
    lbf = np.ascontiguousarray(np.concatenate(
        [(bx[512 + m * P:512 + (m + 1) * P] + bf[m * P:(m + 1) * P])[:, None]
         for m in range(4)], axis=1), np.float32)
    nbias = np.concatenate([bx[0:512] + bs[0:512], bx[512:1024] + bf,
                            bx[1024:1536] + bs[512:1024],
                            bx[1536:2048] + bs[1024:1536]])[None, :].astype(np.float16)

    def rmat(np_):
        r = np.zeros((np_, 4 * np_), np.float16)
        for j in range(np_):
            r[j, 4 * j:4 * j + 4] = 1.0
        return r

    id16 = np.eye(P, dtype=np.float16)
    id32 = np.eye(P, dtype=np.float32)
    ones = np.ones((1, P), np.float16)

    inputs_pad = np.concatenate([inputs, np.zeros((1, IN_DIM), np.float32)], 0)
    in_maps = []
    for c in range(NCORES):
        l6h, colh = _core_heaps(c)
        idx = np.where(colh >= N, N, N - 1 - colh)  # row N = zero pad
        xin = np.ascontiguousarray(inputs_pad[idx].T.astype(np.float16))
        mrow = (l6h < N).astype(np.float32)
        cmask = np.ascontiguousarray(np.tile(mrow[None, :], (P, 1)))
        in_maps.append({
            "xin": xin, "wx": Wx_h, "ws": Ws_h, "wf": Wf_h,
            "lbi": lbi, "lbf": lbf, "nbias": nbias,
            "r32": rmat(32), "r8": rmat(8), "r2": rmat(2), "r4": rmat(4),
            "r1": rmat(1), "id16": id16, "id32": id32, "ones": ones,
            "s32": rmat(32).T.astype(np.float32).copy(),
            "s8": rmat(8).T.astype(np.float32).copy(),
            "s2": rmat(2).T.astype(np.float32).copy(),
            "s4": rmat(4).T.astype(np.float32).copy(),
            "s1": rmat(1).T.astype(np.float32).copy(),
            "cmask": cmask,
        })

    if _NC_CACHE is None:
        _NC_CACHE = _build_program()
    nc = _NC_CACHE

    res = run_bass_kernel_spmd(
        nc, in_maps, list(range(NCORES)),
        trace=bool(os.environ.get("BASS_TRACE")),
    )
    LAST_RESULT = res
    return np.ascontiguousarray(res.results[0]["out"])
